# revision 7
# baseline (speedup 1.0000x reference)
"""AFT-Full kernel for Trainium2, 8 NeuronCores.

Sharding: x [B=8, H=96, W=96, C=512] is split along H (dim 1) into 8 shards
of [8, 12, 96, 512].  Every step of the computation (q/k/v projections,
max over batch, the exp_w_bias matmuls over W, output projection) is local
to an H-slice, so there are no collectives at all.

I/O strategy (the kernel is HBM-bound): both the input and the output
live in HBM as bf16, halving DMA bytes vs f32.  The host pre-transposes
x into c-major layout [h, c, ch, pos] (pos = b*96 + w) so the kernel
needs NO on-device transposes of x.  The output is written bf16 in
p-major layout [h, p, t, c] (pos = t*128 + p) so each partition's 6 KB
is contiguous in HBM; the host un-permutes / upcasts.

Structure per h-row (pos-major: w on partitions everywhere except yT):
  qkv: 4 chained K=128 matmuls per batch + one K=1 bias matmul (ones row
       x [-wq_b | 0 | wv_b]) -> q/k/v biases land in PSUM for free.
       wq is negated in the blob so PSUM q-part = -(q+qb) and
       eq = exp(PSUM) = exp(-(q+qb)) needs no scale and doubles as the
       sigmoid term: y = num * recip((eq+1)*den).
  ek = exp(k) (wk_b cancels in exp(k - max_b k)); emx = max_b ek via one
       strided tensor_reduce; eks_k = ek * recip(emx); ekv = v_psum*eks_k.
  den/num: two N=512 matmuls vs exp(w_bias)^T, den FIRST so the DVE
       recip chain overlaps the num matmul (separate PSUM banks).
  y -> PE transpose (bf16 PSUM) -> yT [65, 768] (row 64 = ones for
       out_b) -> 6 M=128 out-proj matmuls -> copies (split ACT/DVE) ->
       DMA.  Out-proj matmuls 4,5 + their copies are deferred to the
       next iteration so their PSUM slots have been drained.
"""

import sys

if "/opt/trn_rl_repo" not in sys.path:
    sys.path.insert(0, "/opt/trn_rl_repo")

import numpy as np
import ml_dtypes
from contextlib import ExitStack

import concourse.bass as bass
import concourse.bacc as bacc
import concourse.tile as tile
from concourse import masks, mybir
from concourse.bass_utils import run_bass_kernel_spmd

F32 = mybir.dt.float32
I32 = mybir.dt.int32
BF16 = mybir.dt.bfloat16
AF = mybir.ActivationFunctionType
BF16NP = ml_dtypes.bfloat16

B = 8          # batch
S = 96         # H = W = 96
C = 512        # input channels
D = 64         # hidden
HL = 12        # h rows per core
NCORES = 8
P = 128        # partitions
NPOS = B * S   # 768 positions per h
BLOB_F = 1568  # packed weight blob columns

_NC_CACHE = {}


def build_kernel():
    nc = bacc.Bacc()

    xt_d = nc.declare_dram_parameter("xt", [HL, P, 4, NPOS], BF16,
                                     isOutput=False)
    wblob_d = nc.declare_dram_parameter("wblob", [P, BLOB_F], BF16,
                                        isOutput=False)
    out_d = nc.declare_dram_parameter("out", [HL, P, 6, C], BF16,
                                      isOutput=True)

    with tile.TileContext(nc) as tc, ExitStack() as ctx:
        singles = ctx.enter_context(tc.tile_pool(name="singles", bufs=1))

        # ---------------- setup ----------------
        #   cols 0:768      wqkvT   [128, 4(chunk), 192]  (-q|k|v columns)
        #   cols 768:1280   owT+b   [65, 512] (row 64 = out_b)
        #   cols 1280:1376  ewbT    [96, 96]  exp(w_bias)^T
        #   cols 1376:1568  brow    [1, 192]  [-wq_b | 0 | wv_b]
        ident = singles.tile([P, P], BF16)
        masks.make_identity(nc, ident[:])

        # PSUM banks: qkv/nd pool 2x2 + ops pool 4x1 = 8
        qkv_ps = ctx.enter_context(tc.tile_pool(name="qkv", bufs=2, space="PSUM"))
        o_ps = ctx.enter_context(tc.tile_pool(name="ops", bufs=4, space="PSUM"))

        blob = singles.tile([P, BLOB_F], BF16)
        warm = singles.tile([1, 8], BF16)
        ones = singles.tile([1, S], BF16)
        zeros = singles.tile([P, C], BF16)
        nc.gpsimd.memset(warm[:], 0.0)
        nc.scalar.activation(warm[:], warm[:], AF.Exp)
        nc.vector.memset(zeros[:], 0.0)
        nc.vector.memset(ones[:], 1.0)
        nc.sync.dma_start(blob[:], wblob_d[:, :])
        wqkv = blob[:, 0:768].rearrange("p (ch x) -> p ch x", ch=4)
        ow = blob[0:D + 1, 768:1280]
        ewbT = blob[0:S, 1280:1376]
        brow = blob[0:1, 1376:1568]

        # ---------------- main pools ----------------
        xT_pool = ctx.enter_context(tc.tile_pool(name="xT", bufs=3))
        ek_pool = ctx.enter_context(tc.tile_pool(name="ek", bufs=2))
        eq_pool = ctx.enter_context(tc.tile_pool(name="eq", bufs=2))
        small_pool = ctx.enter_context(tc.tile_pool(name="small", bufs=2))
        eks_pool = ctx.enter_context(tc.tile_pool(name="eks", bufs=2))
        den2_pool = ctx.enter_context(tc.tile_pool(name="den2", bufs=2))
        y_pool = ctx.enter_context(tc.tile_pool(name="y", bufs=2))
        yT_pool = ctx.enter_context(tc.tile_pool(name="yT", bufs=2))
        osb_pool = ctx.enter_context(tc.tile_pool(name="osb", bufs=2))

        # warm the PE HAM during the initial DMA ramp (results unused)
        warm_ps = qkv_ps.tile([S, 4, 4 * D], F32, name="warmps", tag="qp")
        for i in range(10):
            nc.tensor.matmul(warm_ps[:, i % 4, :],
                             ident[:, :S], zeros[:, 0:4 * D],
                             start=True, stop=True)

        yT_tiles = [yT_pool.tile([D + 1, NPOS], BF16, tag=f"yt{i}",
                                 name=f"yt{i}") for i in range(2)]
        for t in yT_tiles:
            nc.vector.memset(t[D:D + 1, :], 1.0)

        # ---------------- software-pipelined main loop ----------------
        xT_tiles = {}
        st = {}   # per-h live tiles

        def emit_load(h):
            xT_tiles[h] = xT_pool.tile([P, 4, NPOS], BF16, name=f"xT{h}",
                                       tag="xT")
            nc.sync.dma_start(xT_tiles[h][:], xt_d[h])

        def emit_qkv_half(h, half):
            # (4 K=128 + 1 K=1 bias) matmuls per batch into one 2-bank tile
            xT = xT_tiles[h]
            qp = qkv_ps.tile([S, 4, 4 * D], F32, tag="qp",
                             name=f"qp{h}_{half}")
            for sub in range(4):
                b = half * 4 + sub
                for ch in range(4):
                    nc.tensor.matmul(
                        qp[:, sub, 0:3 * D],
                        xT[:, ch, b * S:(b + 1) * S],
                        wqkv[:, ch, :],
                        start=(ch == 0),
                        stop=False,
                    )
                nc.tensor.matmul(qp[:, sub, 0:3 * D], ones[:], brow[:],
                                 start=False, stop=True)
            return qp

        def emit_outproj(j, trange, yT):
            # out-proj matmuls + ACT/DVE copy + paired DMA drain
            o_sb, ops = st[j, "osb"], st[j, "ops"]
            for t in trange:
                op = o_ps.tile([P, C], F32, tag="op", name=f"op{j}_{t}")
                nc.tensor.matmul(op[:], yT[:, t * P:(t + 1) * P], ow[:],
                                 start=True, stop=True)
                ops[t] = op

        def emit_ocopy(j, t, eng):
            o_sb, ops = st[j, "osb"], st[j, "ops"]
            if eng == "act":
                nc.scalar.copy(o_sb[:, t, :], ops[t][:])
            else:
                nc.vector.tensor_copy(o_sb[:, t, :], ops[t][:])
            ops[t] = None
            st[j, "ocp"].add(t)
            for pair in ((0, 1), (2, 3), (4, 5)):
                if t in pair and all(u in st[j, "ocp"] for u in pair):
                    nc.sync.dma_start(
                        out_d[j][:, pair[0]:pair[1] + 1, :],
                        o_sb[:, pair[0]:pair[1] + 1, :])

        for k in range(HL + 2):
            j = k - 1
            if k == 0:
                emit_load(0)
                emit_load(1)
            if k + 2 < HL:
                emit_load(k + 2)

            if k < HL:
                st[k, "eq"] = eq_pool.tile([S, B, D], BF16, tag="eq", name=f"eq{k}")
                st[k, "ek"] = ek_pool.tile([S, B, D], BF16, tag="ek", name=f"ek{k}")

            # PE: deferred out-proj tail of j-1 (slots drained last iter)
            if k >= 2:
                emit_outproj(k - 2, (4, 5), yT_tiles[k % 2])
            if 1 <= k <= HL:
                # PE: den matmul first, then num (separate banks so the DVE
                # recip chain on den overlaps the num matmul)
                ndp = qkv_ps.tile([S, 2, B * D], F32, tag="qp", name=f"ndp{j}")
                nc.tensor.matmul(ndp[:, 0, :], ewbT[:],
                                 st[j, "eks"][:, :, D:2 * D],
                                 start=True, stop=True)
                nc.tensor.matmul(ndp[:, 1, :], ewbT[:],
                                 st[j, "eks"][:, :, 0:D],
                                 start=True, stop=True)
                den_v = ndp[:, 0, :].rearrange("p (b d) -> p b d", b=B)
                num_v = ndp[:, 1, :].rearrange("p (b d) -> p b d", b=B)
                # DVE: y = num * recip((eq + 1) * den), per batch-half
                den2 = den2_pool.tile([S, B, D], F32, tag="den2", name=f"den{j}")
                rden = den2_pool.tile([S, B, D], F32, tag="rden",
                                      name=f"rden{j}")
                y_sb = y_pool.tile([S, B, D], BF16, tag="y", name=f"y{j}")
                for half in range(2):
                    bsl = slice(half * 4, half * 4 + 4)
                    nc.vector.scalar_tensor_tensor(
                        den2[:, bsl, :], st[j, "eq"][:, bsl, :], 1.0,
                        den_v[:, bsl, :],
                        op0=mybir.AluOpType.add, op1=mybir.AluOpType.mult)
                    nc.vector.reciprocal_approx_fast(
                        rden[:, bsl, :].rearrange("p b d -> p (b d)"),
                        den2[:, bsl, :].rearrange("p b d -> p (b d)"))
                    nc.vector.tensor_mul(y_sb[:, bsl, :], num_v[:, bsl, :],
                                         rden[:, bsl, :])

            # ACT: deferred o-copy 4 of j-1; DVE: o-copy 5 of j-1
            if k >= 2:
                emit_ocopy(k - 2, 4, "act")
                emit_ocopy(k - 2, 5, "dve")

            if k < HL:
                # PE: qkv batches 0-3 ; ACT exps evacuate q and k
                qp0 = emit_qkv_half(k, 0)
                nc.scalar.activation(st[k, "ek"][:, 0:4, :],
                                     qp0[:, :, D:2 * D], AF.Exp)
                nc.scalar.activation(st[k, "eq"][:, 0:4, :],
                                     qp0[:, :, 0:D], AF.Exp)

            if 1 <= k <= HL:
                # PE: transpose y half 0 -> ytp0 (bf16 PSUM via ops pool)
                ytp0 = o_ps.tile([D, 2 * B * D], BF16, tag="op", name=f"yt0_{j}")
                for bb in range(4):
                    nc.tensor.transpose(ytp0[:, bb * S:(bb + 1) * S],
                                        y_sb[:, bb, :], ident[:S, :S])
                yT = yT_tiles[j % 2]
                nc.scalar.copy(yT[0:D, 0:384], ytp0[:, 0:384])

            if k < HL:
                # PE: qkv batches 4-7
                qp1 = emit_qkv_half(k, 1)
                nc.scalar.activation(st[k, "ek"][:, 4:8, :],
                                     qp1[:, :, D:2 * D], AF.Exp)
                nc.scalar.activation(st[k, "eq"][:, 4:8, :],
                                     qp1[:, :, 0:D], AF.Exp)

            if 1 <= k <= HL:
                # PE: transpose y half 1; DVE copies ytp1 -> yT
                ytp1 = o_ps.tile([D, 2 * B * D], BF16, tag="op", name=f"yt1_{j}")
                for bb in range(4):
                    b = 4 + bb
                    nc.tensor.transpose(ytp1[:, bb * S:(bb + 1) * S],
                                        y_sb[:, b, :], ident[:S, :S])
                nc.vector.tensor_copy(yT[0:D, 384:768], ytp1[:, 0:384])

                # PE: out-proj tiles 0-3 (4,5 deferred to next iteration)
                st[j, "osb"] = osb_pool.tile([P, 6, C], BF16, tag="osb",
                                             name=f"osb{j}")
                st[j, "ops"] = [None] * 6
                st[j, "ocp"] = set()
                emit_outproj(j, (0, 1, 2, 3), yT)

            if k < HL:
                # DVE: emx = max_b ek (strided reduce); eks_k = ek*recip(emx)
                ek_raw = st[k, "ek"]
                emx = small_pool.tile([S, D], F32, tag="emx")
                remx = small_pool.tile([S, D], F32, tag="remx")
                nc.vector.tensor_reduce(
                    emx[:], ek_raw[:].rearrange("p b d -> p d b"),
                    mybir.AxisListType.X, mybir.AluOpType.max)
                nc.vector.reciprocal_approx_fast(remx[:], emx[:])

                eks = eks_pool.tile([S, B, 2 * D], BF16, tag="eks", name=f"eks{k}")
                st[k, "eks"] = eks
                nc.vector.tensor_mul(
                    eks[:, :, D:2 * D], ek_raw[:],
                    remx[:, :].rearrange("p (o d) -> p o d", o=1).broadcast_to([S, B, D]))
                # ekv = v_psum * eks_k  (v bias already added by the bias MM)
                nc.vector.scalar_tensor_tensor(
                    eks[:, 0:4, 0:D], qp0[:, :, 2 * D:3 * D], 1.0,
                    eks[:, 0:4, D:2 * D],
                    op0=mybir.AluOpType.mult, op1=mybir.AluOpType.mult)
                nc.vector.scalar_tensor_tensor(
                    eks[:, 4:8, 0:D], qp1[:, :, 2 * D:3 * D], 1.0,
                    eks[:, 4:8, D:2 * D],
                    op0=mybir.AluOpType.mult, op1=mybir.AluOpType.mult)

            if 1 <= k <= HL:
                # ACT/DVE: evacuate out-proj tiles 0-3
                emit_ocopy(j, 0, "act")
                emit_ocopy(j, 1, "dve")
                emit_ocopy(j, 2, "act")
                emit_ocopy(j, 3, "dve")

            if k >= 2:
                for key in ("eq", "ek", "eks", "osb", "ops", "ocp"):
                    st.pop((k - 2, key), None)
                xT_tiles.pop(k - 2, None)

    if not nc.is_finalized():
        nc.finalize()
    return nc


def _make_blob(wq_w, wq_b, wk_w, wk_b, wv_w, wv_b, out_w, out_b, w_bias_table):
    blob = np.zeros((P, BLOB_F), dtype=np.float32)
    # wq negated so exp(-(q+qb)) is a scale=1 activation
    for j, w in enumerate([-np.asarray(wq_w), wk_w, wv_w]):  # wqkvT [128, 4, 192]
        w = np.asarray(w)
        for ch in range(4):
            blob[:, ch * 192 + j * D:(ch * 192 + (j + 1) * D)] = \
                w[:, ch * P:(ch + 1) * P].T
    blob[0:D, 768:1280] = np.asarray(out_w).T        # owT
    blob[D, 768:1280] = out_b
    blob[0:S, 1280:1376] = np.exp(np.asarray(w_bias_table)).T
    # bias row for the K=1 matmul: [-wq_b | 0 | wv_b]
    # (wk_b cancels exactly in exp(k - max_b k))
    blob[0, 1376:1440] = -np.asarray(wq_b)
    blob[0, 1504:1568] = np.asarray(wv_b)
    return blob


def _build_in_maps(x, wq_w, wq_b, wk_w, wk_b, wv_w, wv_b, out_w, out_b,
                   w_bias_table):
    blob = _make_blob(wq_w, wq_b, wk_w, wk_b, wv_w, wv_b, out_w, out_b,
                      w_bias_table).astype(BF16NP)
    xbf = np.asarray(x).astype(BF16NP)               # [8, 96, 96, 512]
    in_maps = []
    for i in range(NCORES):
        # [B, HL, S, C] -> [HL, C, B, S] -> [HL, 4, 128, B*S]
        # -> [HL, 128, 4, B*S]  (c-within-chunk on partitions)
        xs = xbf[:, i * HL:(i + 1) * HL].transpose(1, 3, 0, 2)
        xs = xs.reshape(HL, 4, P, NPOS).transpose(0, 2, 1, 3)
        in_maps.append({
            "wblob": blob,
            "xt": np.ascontiguousarray(xs),
        })
    return in_maps


def kernel(x, wq_w, wq_b, wk_w, wk_b, wv_w, wv_b, out_w, out_b, w_bias_table):
    if "nc" not in _NC_CACHE:
        _NC_CACHE["nc"] = build_kernel()
    nc = _NC_CACHE["nc"]

    in_maps = _build_in_maps(x, wq_w, wq_b, wk_w, wk_b, wv_w, wv_b,
                             out_w, out_b, w_bias_table)
    res = run_bass_kernel_spmd(nc, in_maps, list(range(NCORES)))
    # per-core out: [HL, P, 6, C] bf16, pos = t*128 + p = b*96 + w
    outs = []
    for i in range(NCORES):
        a = np.asarray(res.results[i]["out"])        # [HL, 128, 6, 512]
        a = a.transpose(0, 2, 1, 3).reshape(HL, B, S, C).transpose(1, 0, 2, 3)
        outs.append(a)
    return np.concatenate(outs, axis=1).astype(np.float32)


# revision 8
# speedup vs baseline: 1.4729x; 1.4729x over previous
"""AFT-Full kernel for Trainium2, 8 NeuronCores.

Sharding: x [B=8, H=96, W=96, C=512] is split along H (dim 1) into 8 shards
of [8, 12, 96, 512].  Every step of the computation (q/k/v projections,
max over batch, the exp_w_bias matmuls over W, output projection) is local
to an H-slice, so there are no collectives at all.

I/O strategy (the kernel is HBM-bound): both the input and the output
live in HBM as bf16, halving DMA bytes vs f32.  The host pre-transposes
x into c-major layout [h, c, ch, pos] (pos = b*96 + w) so the kernel
needs NO on-device transposes of x.  The output is written bf16 in
p-major layout [h, p, t, c] (pos = t*128 + p) so each partition's 6 KB
is contiguous in HBM; the host un-permutes / upcasts.

Structure per h-row (pos-major: w on partitions everywhere except yT):
  qkv: 4 chained K=128 matmuls per batch; wq negated in the blob so
       eq = exp(PSUM q) = exp(-q); eq*exp(-wq_b) doubles as the sigmoid
       term teq: y = num * recip((teq+1)*den).  One merged exp evacuates
       q|k (adjacent PSUM columns).
  ek = exp(k) (wk_b cancels in exp(k - max_b k)); emx = max_b ek via a
       max tree; eks_k = ek * recip(emx); ekv = eks_k * (v + wv_b).
  den/num: two N=512 matmuls vs exp(w_bias)^T with contiguous rhs
       (eks laid out [S, 2, B, D]), den FIRST so the DVE recip chain
       overlaps the num matmul (separate PSUM banks).
  y -> PE transpose (bf16 PSUM) -> yT [65, 768] (row 64 = ones for
       out_b) -> 6 M=128 out-proj matmuls -> copies (split ACT/DVE) ->
       DMA.  Out-proj matmuls 4,5 + their copies are deferred to the
       next iteration so their PSUM slots have been drained.
"""

import sys

if "/opt/trn_rl_repo" not in sys.path:
    sys.path.insert(0, "/opt/trn_rl_repo")

import numpy as np
import ml_dtypes
from contextlib import ExitStack

import concourse.bass as bass
import concourse.bacc as bacc
import concourse.tile as tile
from concourse import masks, mybir
from concourse.bass_utils import run_bass_kernel_spmd

F32 = mybir.dt.float32
I32 = mybir.dt.int32
BF16 = mybir.dt.bfloat16
AF = mybir.ActivationFunctionType
BF16NP = ml_dtypes.bfloat16

B = 8          # batch
S = 96         # H = W = 96
C = 512        # input channels
D = 64         # hidden
HL = 12        # h rows per core
NCORES = 8
P = 128        # partitions
NPOS = B * S   # 768 positions per h
BLOB_F = 2400  # packed weight blob columns

_NC_CACHE = {}


def build_kernel():
    nc = bacc.Bacc()

    xt_d = nc.declare_dram_parameter("xt", [HL, P, 4, NPOS], BF16,
                                     isOutput=False)
    wblob_d = nc.declare_dram_parameter("wblob", [P, BLOB_F], BF16,
                                        isOutput=False)
    out_d = nc.declare_dram_parameter("out", [HL, P, 6, C], BF16,
                                      isOutput=True)

    with tile.TileContext(nc) as tc, ExitStack() as ctx:
        singles = ctx.enter_context(tc.tile_pool(name="singles", bufs=1))

        # ---------------- setup ----------------
        #   cols 0:768      wqkvT   [128, 4(chunk), 192]  (-q|k|v columns)
        #   cols 768:1280   owT+b   [65, 512] (row 64 = out_b)
        #   cols 1280:1376  ewbT    [96, 96]  exp(w_bias)^T
        #   cols 1376:1888  eqb8    [96, 512] exp(-wq_b) tiled 8x (b,d)
        #   cols 1888:2400  vb8     [96, 512] wv_b tiled 8x (b,d)
        ident = singles.tile([P, P], BF16)
        masks.make_identity(nc, ident[:])

        # PSUM banks: qkv/nd pool 2x2 + ops pool 4x1 = 8
        qkv_ps = ctx.enter_context(tc.tile_pool(name="qkv", bufs=2, space="PSUM"))
        o_ps = ctx.enter_context(tc.tile_pool(name="ops", bufs=4, space="PSUM"))

        blob = singles.tile([P, BLOB_F], BF16)
        warm = singles.tile([1, 8], BF16)
        zeros = singles.tile([P, C], BF16)
        nc.gpsimd.memset(warm[:], 0.0)
        nc.scalar.activation(warm[:], warm[:], AF.Exp)
        nc.vector.memset(zeros[:], 0.0)
        nc.sync.dma_start(blob[:], wblob_d[:, :])
        wqkv = blob[:, 0:768].rearrange("p (ch x) -> p ch x", ch=4)
        ow = blob[0:D + 1, 768:1280]
        ewbT = blob[0:S, 1280:1376]
        eqb8 = blob[0:S, 1376:1888].rearrange("p (b d) -> p b d", b=B)
        vb8 = blob[0:S, 1888:2400].rearrange("p (b d) -> p b d", b=B)

        # ---------------- main pools ----------------
        xT_pool = ctx.enter_context(tc.tile_pool(name="xT", bufs=3))
        eqk_pool = ctx.enter_context(tc.tile_pool(name="eqk", bufs=2))
        v_pool = ctx.enter_context(tc.tile_pool(name="v", bufs=2))
        teq_pool = ctx.enter_context(tc.tile_pool(name="teq", bufs=2))
        small_pool = ctx.enter_context(tc.tile_pool(name="small", bufs=2))
        eks_pool = ctx.enter_context(tc.tile_pool(name="eks", bufs=2))
        den2_pool = ctx.enter_context(tc.tile_pool(name="den2", bufs=2))
        y_pool = ctx.enter_context(tc.tile_pool(name="y", bufs=2))
        yT_pool = ctx.enter_context(tc.tile_pool(name="yT", bufs=2))
        osb_pool = ctx.enter_context(tc.tile_pool(name="osb", bufs=2))

        # warm the PE HAM during the initial DMA ramp (results unused)
        warm_ps = qkv_ps.tile([S, 4, 4 * D], F32, name="warmps", tag="qp")
        for i in range(10):
            nc.tensor.matmul(warm_ps[:, i % 4, :],
                             ident[:, :S], zeros[:, 0:4 * D],
                             start=True, stop=True)

        yT_tiles = [yT_pool.tile([D + 1, NPOS], BF16, tag=f"yt{i}",
                                 name=f"yt{i}") for i in range(2)]
        for t in yT_tiles:
            nc.vector.memset(t[D:D + 1, :], 1.0)

        # ---------------- software-pipelined main loop ----------------
        xT_tiles = {}
        st = {}   # per-h live tiles

        def emit_load(h):
            xT_tiles[h] = xT_pool.tile([P, 4, NPOS], BF16, name=f"xT{h}",
                                       tag="xT")
            nc.sync.dma_start(xT_tiles[h][:], xt_d[h])

        def emit_qkv_half(h, half):
            # 16 matmuls for batches 4*half..4*half+3 into one 2-bank tile
            xT = xT_tiles[h]
            qp = qkv_ps.tile([S, 4, 4 * D], F32, tag="qp",
                             name=f"qp{h}_{half}")
            for sub in range(4):
                b = half * 4 + sub
                for ch in range(4):
                    nc.tensor.matmul(
                        qp[:, sub, 0:3 * D],
                        xT[:, ch, b * S:(b + 1) * S],
                        wqkv[:, ch, :],
                        start=(ch == 0),
                        stop=(ch == 3),
                    )
            return qp

        def emit_outproj(j, trange, yT):
            o_sb, ops = st[j, "osb"], st[j, "ops"]
            for t in trange:
                op = o_ps.tile([P, C], F32, tag="op", name=f"op{j}_{t}")
                nc.tensor.matmul(op[:], yT[:, t * P:(t + 1) * P], ow[:],
                                 start=True, stop=True)
                ops[t] = op

        def emit_ocopy(j, t, eng):
            o_sb, ops = st[j, "osb"], st[j, "ops"]
            if eng == "act":
                nc.scalar.copy(o_sb[:, t, :], ops[t][:])
            else:
                nc.vector.tensor_copy(o_sb[:, t, :], ops[t][:])
            ops[t] = None
            st[j, "ocp"].add(t)
            for pair in ((0, 1), (2, 3), (4, 5)):
                if t in pair and all(u in st[j, "ocp"] for u in pair):
                    nc.sync.dma_start(
                        out_d[j][:, pair[0]:pair[1] + 1, :],
                        o_sb[:, pair[0]:pair[1] + 1, :])

        for k in range(HL + 2):
            j = k - 1
            if k == 0:
                emit_load(0)
                emit_load(1)
            if k + 2 < HL:
                emit_load(k + 2)

            if k < HL:
                # eqk holds [exp(-q) | exp(k)] so one ACT op evacuates both
                st[k, "eqk"] = eqk_pool.tile([S, B, 2 * D], BF16, tag="eqk",
                                             name=f"eqk{k}")
                st[k, "v"] = v_pool.tile([S, B, D], BF16, tag="v", name=f"v{k}")

            # PE: deferred out-proj tail of j-1 (slots drained last iter)
            if k >= 2:
                emit_outproj(k - 2, (4, 5), yT_tiles[k % 2])
            if 1 <= k <= HL:
                # PE: den matmul first, then num (separate banks so the DVE
                # recip chain on den overlaps the num matmul)
                ndp = qkv_ps.tile([S, 2, B * D], F32, tag="qp", name=f"ndp{j}")
                eksj = st[j, "eks"]
                nc.tensor.matmul(ndp[:, 0, :], ewbT[:], eksj[:, 1, :, :],
                                 start=True, stop=True)
                nc.tensor.matmul(ndp[:, 1, :], ewbT[:], eksj[:, 0, :, :],
                                 start=True, stop=True)
                den_v = ndp[:, 0, :].rearrange("p (b d) -> p b d", b=B)
                num_v = ndp[:, 1, :].rearrange("p (b d) -> p b d", b=B)
                # DVE: y = num * recip((teq + 1) * den)
                den2 = den2_pool.tile([S, B, D], F32, tag="den2", name=f"den{j}")
                rden = den2_pool.tile([S, B, D], F32, tag="rden",
                                      name=f"rden{j}")
                y_sb = y_pool.tile([S, B, D], BF16, tag="y", name=f"y{j}")
                nc.vector.scalar_tensor_tensor(
                    den2[:], st[j, "teq"][:], 1.0, den_v[:],
                    op0=mybir.AluOpType.add, op1=mybir.AluOpType.mult)
                nc.vector.reciprocal_approx_fast(
                    rden[:].rearrange("p b d -> p (b d)"),
                    den2[:].rearrange("p b d -> p (b d)"))
                nc.vector.tensor_mul(y_sb[:], num_v[:], rden[:])

            # ACT: deferred o-copies 4,5 of j-1
            if k >= 2:
                emit_ocopy(k - 2, 4, "act")
                emit_ocopy(k - 2, 5, "act")

            if k < HL:
                # PE: qkv batches 0-3 ; ACT: merged exp for q|k
                qp0 = emit_qkv_half(k, 0)
                nc.scalar.activation(st[k, "eqk"][:, 0:4, :],
                                     qp0[:, :, 0:2 * D], AF.Exp)
                ek_v = st[k, "eqk"][:, :, D:2 * D]
                # DVE: partial max over batches 0-3
                mxA = small_pool.tile([S, 2, D], BF16, tag="mxA")
                nc.vector.tensor_max(mxA[:], ek_v[:, 0:2, :], ek_v[:, 2:4, :])
                st[k, "mxA"] = mxA

            if 1 <= k <= HL:
                # PE: transpose y half 0 -> ytp0 (bf16 PSUM via ops pool)
                ytp0 = o_ps.tile([D, 2 * B * D], BF16, tag="op", name=f"yt0_{j}")
                for bb in range(4):
                    nc.tensor.transpose(ytp0[:, bb * S:(bb + 1) * S],
                                        y_sb[:, bb, :], ident[:S, :S])
                yT = yT_tiles[j % 2]
                nc.scalar.copy(yT[0:D, 0:384], ytp0[:, 0:384])

            if k < HL:
                # PE: qkv batches 4-7
                qp1 = emit_qkv_half(k, 1)
                nc.scalar.activation(st[k, "eqk"][:, 4:8, :],
                                     qp1[:, :, 0:2 * D], AF.Exp)

            if 1 <= k <= HL:
                # PE: transpose y half 1; DVE copies ytp1 -> yT
                ytp1 = o_ps.tile([D, 2 * B * D], BF16, tag="op", name=f"yt1_{j}")
                for bb in range(4):
                    b = 4 + bb
                    nc.tensor.transpose(ytp1[:, bb * S:(bb + 1) * S],
                                        y_sb[:, b, :], ident[:S, :S])
                nc.vector.tensor_copy(yT[0:D, 384:768], ytp1[:, 0:384])

                # PE: out-proj tiles 0-3 (4,5 deferred to next iteration)
                st[j, "osb"] = osb_pool.tile([P, 6, C], BF16, tag="osb",
                                             name=f"osb{j}")
                st[j, "ops"] = [None] * 6
                st[j, "ocp"] = set()
                emit_outproj(j, (0, 1, 2, 3), yT)

            if k < HL:
                # DVE: v-bias add; finish max tree; eks (layout [S, 2, B, D]:
                # block 0 = ekv, block 1 = ek_scaled -> contiguous nd rhs)
                eqk = st[k, "eqk"]
                ek_v = eqk[:, :, D:2 * D]
                nc.vector.tensor_add(st[k, "v"][:, 0:4, :],
                                     qp0[:, :, 2 * D:3 * D], vb8[:, 0:4, :])
                nc.vector.tensor_add(st[k, "v"][:, 4:8, :],
                                     qp1[:, :, 2 * D:3 * D], vb8[:, 4:8, :])
                mxB = small_pool.tile([S, 2, D], BF16, tag="mxB")
                mx2 = small_pool.tile([S, 2, D], BF16, tag="mx2")
                emx = small_pool.tile([S, D], F32, tag="emx")
                remx = small_pool.tile([S, D], F32, tag="remx")
                nc.vector.tensor_max(mxB[:], ek_v[:, 4:6, :], ek_v[:, 6:8, :])
                nc.vector.tensor_max(mx2[:], st[k, "mxA"][:], mxB[:])
                nc.vector.tensor_max(
                    emx[:], mx2[:, 0:1, :].rearrange("p o d -> p (o d)"),
                    mx2[:, 1:2, :].rearrange("p o d -> p (o d)"))
                nc.vector.reciprocal_approx_fast(remx[:], emx[:])

                eks = eks_pool.tile([S, 2, B, D], BF16, tag="eks", name=f"eks{k}")
                st[k, "eks"] = eks
                nc.vector.tensor_mul(
                    eks[:, 1, :, :], ek_v[:],
                    remx[:, :].rearrange("p (o d) -> p o d", o=1).broadcast_to([S, B, D]))
                nc.vector.tensor_mul(eks[:, 0, :, :], eks[:, 1, :, :],
                                     st[k, "v"][:])
                teq = teq_pool.tile([S, B, D], BF16, tag="teq", name=f"teq{k}")
                nc.vector.tensor_mul(teq[:], eqk[:, :, 0:D], eqb8[:])
                st[k, "teq"] = teq

            if 1 <= k <= HL:
                # ACT/DVE: evacuate out-proj tiles 0-3
                emit_ocopy(j, 0, "act")
                emit_ocopy(j, 1, "dve")
                emit_ocopy(j, 2, "act")
                emit_ocopy(j, 3, "dve")

            if k >= 2:
                for key in ("eqk", "v", "teq", "eks", "mxA", "osb", "ops", "ocp"):
                    st.pop((k - 2, key), None)
                xT_tiles.pop(k - 2, None)

    if not nc.is_finalized():
        nc.finalize()
    return nc


def _make_blob(wq_w, wq_b, wk_w, wk_b, wv_w, wv_b, out_w, out_b, w_bias_table):
    blob = np.zeros((P, BLOB_F), dtype=np.float32)
    # wq negated so exp(-q) is a scale=1 activation
    for j, w in enumerate([-np.asarray(wq_w), wk_w, wv_w]):  # wqkvT [128, 4, 192]
        w = np.asarray(w)
        for ch in range(4):
            blob[:, ch * 192 + j * D:(ch * 192 + (j + 1) * D)] = \
                w[:, ch * P:(ch + 1) * P].T
    blob[0:D, 768:1280] = np.asarray(out_w).T        # owT
    blob[D, 768:1280] = out_b
    blob[0:S, 1280:1376] = np.exp(np.asarray(w_bias_table)).T
    # wk_b cancels exactly in exp(k - max_b k); wq_b folded via exp(-wq_b),
    # wv_b added to v after the projection.
    blob[0:S, 1376:1888] = np.tile(np.exp(-np.asarray(wq_b)), B)[None, :]
    blob[0:S, 1888:2400] = np.tile(np.asarray(wv_b), B)[None, :]
    return blob


def _build_in_maps(x, wq_w, wq_b, wk_w, wk_b, wv_w, wv_b, out_w, out_b,
                   w_bias_table):
    blob = _make_blob(wq_w, wq_b, wk_w, wk_b, wv_w, wv_b, out_w, out_b,
                      w_bias_table).astype(BF16NP)
    xbf = np.asarray(x).astype(BF16NP)               # [8, 96, 96, 512]
    in_maps = []
    for i in range(NCORES):
        # [B, HL, S, C] -> [HL, C, B, S] -> [HL, 4, 128, B*S]
        # -> [HL, 128, 4, B*S]  (c-within-chunk on partitions)
        xs = xbf[:, i * HL:(i + 1) * HL].transpose(1, 3, 0, 2)
        xs = xs.reshape(HL, 4, P, NPOS).transpose(0, 2, 1, 3)
        in_maps.append({
            "wblob": blob,
            "xt": np.ascontiguousarray(xs),
        })
    return in_maps


def kernel(x, wq_w, wq_b, wk_w, wk_b, wv_w, wv_b, out_w, out_b, w_bias_table):
    if "nc" not in _NC_CACHE:
        _NC_CACHE["nc"] = build_kernel()
    nc = _NC_CACHE["nc"]

    in_maps = _build_in_maps(x, wq_w, wq_b, wk_w, wk_b, wv_w, wv_b,
                             out_w, out_b, w_bias_table)
    res = run_bass_kernel_spmd(nc, in_maps, list(range(NCORES)))
    # per-core out: [HL, P, 6, C] bf16, pos = t*128 + p = b*96 + w
    outs = []
    for i in range(NCORES):
        a = np.asarray(res.results[i]["out"])        # [HL, 128, 6, 512]
        a = a.transpose(0, 2, 1, 3).reshape(HL, B, S, C).transpose(1, 0, 2, 3)
        outs.append(a)
    return np.concatenate(outs, axis=1).astype(np.float32)


# revision 13
# speedup vs baseline: 2.0760x; 1.4095x over previous
"""AFT-Full kernel for Trainium2, 8 NeuronCores.

Sharding: x [B=8, H=96, W=96, C=512] is split along H (dim 1) into 8 shards
of [8, 12, 96, 512].  Every step of the computation (q/k/v projections,
max over batch, the exp_w_bias matmuls over W, output projection) is local
to an H-slice, so there are no collectives at all.

I/O strategy (the kernel is HBM-bound): both the input and the output
live in HBM as bf16, halving DMA bytes vs f32.  The host pre-transposes
x into c-major layout [h, c, ch, pos] (pos = b*96 + w) so the kernel
needs NO on-device transposes of x.  The output is written bf16 in
p-major layout [h, p, t, c] (pos = t*128 + p) so each partition's 6 KB
is contiguous in HBM; the host un-permutes / upcasts.

THREE-stage software pipeline (emission order per engine = schedule):
  stage A (iter k):   qkv matmuls + exp evac + max tree + eks/teq tail.
                      The DVE tail may slide into iter k+1 freely -- its
                      consumer (stage B of the same h) only runs in k+2.
  stage B (iter k+2): den/num matmuls (den first; separate banks), DVE
                      recip chain -> y -> PE transposes -> yT -> out-proj
                      matmuls 0-3 + copies.
  stage C (iter k+3): out-proj matmuls 4,5 + copies + final DMA (their
                      PSUM slots are guaranteed drained by then).
This keeps every cross-iteration dependency slack >= 1 full iteration,
so no engine ever stalls long enough to drop the PE HAM clock.
"""

import sys

if "/opt/trn_rl_repo" not in sys.path:
    sys.path.insert(0, "/opt/trn_rl_repo")

import numpy as np
import ml_dtypes
from contextlib import ExitStack

import concourse.bass as bass
import concourse.bacc as bacc
import concourse.tile as tile
from concourse import masks, mybir
from concourse.bass_utils import run_bass_kernel_spmd

F32 = mybir.dt.float32
I32 = mybir.dt.int32
BF16 = mybir.dt.bfloat16
AF = mybir.ActivationFunctionType
BF16NP = ml_dtypes.bfloat16

B = 8          # batch
S = 96         # H = W = 96
C = 512        # input channels
D = 64         # hidden
HL = 12        # h rows per core
NCORES = 8
P = 128        # partitions
NPOS = B * S   # 768 positions per h
BLOB_F = 2400  # packed weight blob columns

_NC_CACHE = {}


def build_kernel():
    nc = bacc.Bacc()

    xt_d = nc.declare_dram_parameter("xt", [HL, P, 4, NPOS], BF16,
                                     isOutput=False)
    wblob_d = nc.declare_dram_parameter("wblob", [P, BLOB_F], BF16,
                                        isOutput=False)
    out_d = nc.declare_dram_parameter("out", [HL, P, 6, C], BF16,
                                      isOutput=True)

    with tile.TileContext(nc) as tc, ExitStack() as ctx:
        singles = ctx.enter_context(tc.tile_pool(name="singles", bufs=1))

        # ---------------- setup ----------------
        #   cols 0:768      wqkvT   [128, 4(chunk), 192]  (-q|k|v columns)
        #   cols 768:1280   owT+b   [65, 512] (row 64 = out_b)
        #   cols 1280:1376  ewbT    [96, 96]  exp(w_bias)^T
        #   cols 1376:1888  eqb8    [96, 512] exp(-wq_b) tiled 8x (b,d)
        #   cols 1888:2400  vb8     [96, 512] wv_b tiled 8x (b,d)
        ident = singles.tile([P, P], BF16)
        masks.make_identity(nc, ident[:])

        # PSUM banks: qkv/nd pool 2x2 + ops pool 4x1 = 8
        qkv_ps = ctx.enter_context(tc.tile_pool(name="qkv", bufs=2, space="PSUM"))
        o_ps = ctx.enter_context(tc.tile_pool(name="ops", bufs=4, space="PSUM"))

        blob = singles.tile([P, BLOB_F], BF16)
        warm = singles.tile([1, 8], BF16)
        zeros = singles.tile([P, C], BF16)
        nc.gpsimd.memset(warm[:], 0.0)
        nc.scalar.activation(warm[:], warm[:], AF.Exp)
        nc.vector.memset(zeros[:], 0.0)
        nc.sync.dma_start(blob[:], wblob_d[:, :])
        wqkv = blob[:, 0:768].rearrange("p (ch x) -> p ch x", ch=4)
        ow = blob[0:D + 1, 768:1280]
        ewbT = blob[0:S, 1280:1376]
        eqb8 = blob[0:S, 1376:1888].rearrange("p (b d) -> p b d", b=B)
        vb8 = blob[0:S, 1888:2400].rearrange("p (b d) -> p b d", b=B)

        # ---------------- main pools ----------------
        xT_pool = ctx.enter_context(tc.tile_pool(name="xT", bufs=3))
        eqk_pool = ctx.enter_context(tc.tile_pool(name="eqk", bufs=2))
        v_pool = ctx.enter_context(tc.tile_pool(name="v", bufs=2))
        teq_pool = ctx.enter_context(tc.tile_pool(name="teq", bufs=3))
        small_pool = ctx.enter_context(tc.tile_pool(name="small", bufs=2))
        eks_pool = ctx.enter_context(tc.tile_pool(name="eks", bufs=3))
        den2_pool = ctx.enter_context(tc.tile_pool(name="den2", bufs=2))
        y_pool = ctx.enter_context(tc.tile_pool(name="y", bufs=2))
        yT_pool = ctx.enter_context(tc.tile_pool(name="yT", bufs=2))
        osb_pool = ctx.enter_context(tc.tile_pool(name="osb", bufs=2))

        # warm the PE HAM during the initial DMA ramp (results unused)
        warm_ps = qkv_ps.tile([S, 4, 4 * D], F32, name="warmps", tag="qp")
        for i in range(10):
            nc.tensor.matmul(warm_ps[:, i % 4, :],
                             ident[:, :S], zeros[:, 0:4 * D],
                             start=True, stop=True)

        yT_tiles = [yT_pool.tile([D + 1, NPOS], BF16, tag=f"yt{i}",
                                 name=f"yt{i}") for i in range(2)]
        for t in yT_tiles:
            nc.vector.memset(t[D:D + 1, :], 1.0)

        # ---------------- software-pipelined main loop ----------------
        xT_tiles = {}
        st = {}   # per-h live tiles

        def emit_load(h):
            xT_tiles[h] = xT_pool.tile([P, 4, NPOS], BF16, name=f"xT{h}",
                                       tag="xT")
            nc.sync.dma_start(xT_tiles[h][:], xt_d[h])

        def emit_qkv_half(h, half):
            # 16 matmuls for batches 4*half..4*half+3 into one 2-bank tile
            xT = xT_tiles[h]
            qp = qkv_ps.tile([S, 4, 4 * D], F32, tag="qp",
                             name=f"qp{h}_{half}")
            for sub in range(4):
                b = half * 4 + sub
                for ch in range(4):
                    nc.tensor.matmul(
                        qp[:, sub, 0:3 * D],
                        xT[:, ch, b * S:(b + 1) * S],
                        wqkv[:, ch, :],
                        start=(ch == 0),
                        stop=(ch == 3),
                    )
            return qp

        def emit_outproj(j, trange):
            o_sb, ops = st[j, "osb"], st[j, "ops"]
            yT = yT_tiles[j % 2]
            for t in trange:
                op = o_ps.tile([P, C], F32, tag="op", name=f"op{j}_{t}")
                nc.tensor.matmul(op[:], yT[:, t * P:(t + 1) * P], ow[:],
                                 start=True, stop=True)
                ops[t] = op

        def emit_ocopy(j, t, eng):
            o_sb, ops = st[j, "osb"], st[j, "ops"]
            if eng == "act":
                nc.scalar.copy(o_sb[:, t, :], ops[t][:])
            else:
                nc.vector.tensor_copy(o_sb[:, t, :], ops[t][:])
            ops[t] = None
            st[j, "ocp"].add(t)
            for pair in ((0, 1), (2, 3), (4, 5)):
                if t in pair and all(u in st[j, "ocp"] for u in pair):
                    nc.sync.dma_start(
                        out_d[j][:, pair[0]:pair[1] + 1, :],
                        o_sb[:, pair[0]:pair[1] + 1, :])

        NIT = HL + 3
        for k in range(NIT):
            j = k - 2    # stage-B index
            j3 = k - 3   # stage-C index
            if k == 0:
                emit_load(0)
                emit_load(1)
            if k + 2 < HL:
                emit_load(k + 2)

            if k < HL:
                # eqk holds [exp(-q) | exp(k)] so one ACT op evacuates both
                st[k, "eqk"] = eqk_pool.tile([S, B, 2 * D], BF16, tag="eqk",
                                             name=f"eqk{k}")
                st[k, "v"] = v_pool.tile([S, B, D], BF16, tag="v", name=f"v{k}")

            # ---- stage C: deferred out-proj tail of j3 ----
            if 0 <= j3:
                emit_outproj(j3, (4, 5))
                emit_ocopy(j3, 4, "act")
                emit_ocopy(j3, 5, "act")

            # ---- stage B: den/num + y chain for j ----
            if 0 <= j < HL:
                ndp = qkv_ps.tile([S, 2, B * D], F32, tag="qp", name=f"ndp{j}")
                eksj = st[j, "eks"]
                nc.tensor.matmul(ndp[:, 0, :], ewbT[:], eksj[:, 1, :, :],
                                 start=True, stop=True)
                nc.tensor.matmul(ndp[:, 1, :], ewbT[:], eksj[:, 0, :, :],
                                 start=True, stop=True)
                den_v = ndp[:, 0, :].rearrange("p (b d) -> p b d", b=B)
                num_v = ndp[:, 1, :].rearrange("p (b d) -> p b d", b=B)
                # DVE: y = num * recip((teq + 1) * den)
                den2 = den2_pool.tile([S, B, D], F32, tag="den2", name=f"den{j}")
                rden = den2_pool.tile([S, B, D], F32, tag="rden",
                                      name=f"rden{j}")
                y_sb = y_pool.tile([S, B, D], BF16, tag="y", name=f"y{j}")
                nc.vector.scalar_tensor_tensor(
                    den2[:], st[j, "teq"][:], 1.0, den_v[:],
                    op0=mybir.AluOpType.add, op1=mybir.AluOpType.mult)
                nc.vector.reciprocal_approx_fast(
                    rden[:].rearrange("p b d -> p (b d)"),
                    den2[:].rearrange("p b d -> p (b d)"))
                nc.vector.tensor_mul(y_sb[:], num_v[:], rden[:])

            # ---- stage A: qkv half 0 ----
            if k < HL:
                qp0 = emit_qkv_half(k, 0)
                nc.scalar.activation(st[k, "eqk"][:, 0:4, :],
                                     qp0[:, :, 0:2 * D], AF.Exp)
                nc.vector.tensor_add(st[k, "v"][:, 0:4, :],
                                     qp0[:, :, 2 * D:3 * D], vb8[:, 0:4, :])
                ek_v0 = st[k, "eqk"][:, :, D:2 * D]
                mxA = small_pool.tile([S, 2, D], BF16, tag="mxA")
                nc.vector.tensor_max(mxA[:], ek_v0[:, 0:2, :], ek_v0[:, 2:4, :])
                st[k, "mxA"] = mxA

            # ---- stage B: transposes half 0 ----
            if 0 <= j < HL:
                ytp0 = o_ps.tile([D, 2 * B * D], BF16, tag="op", name=f"yt0_{j}")
                for bb in range(4):
                    nc.tensor.transpose(ytp0[:, bb * S:(bb + 1) * S],
                                        y_sb[:, bb, :], ident[:S, :S])
                yT = yT_tiles[j % 2]
                nc.scalar.copy(yT[0:D, 0:384], ytp0[:, 0:384])

            # ---- stage A: qkv half 1 ----
            if k < HL:
                qp1 = emit_qkv_half(k, 1)
                nc.scalar.activation(st[k, "eqk"][:, 4:8, :],
                                     qp1[:, :, 0:2 * D], AF.Exp)
                nc.vector.tensor_add(st[k, "v"][:, 4:8, :],
                                     qp1[:, :, 2 * D:3 * D], vb8[:, 4:8, :])

            # ---- stage B: transposes half 1 + out-proj 0-3 ----
            if 0 <= j < HL:
                ytp1 = o_ps.tile([D, 2 * B * D], BF16, tag="op", name=f"yt1_{j}")
                for bb in range(4):
                    b = 4 + bb
                    nc.tensor.transpose(ytp1[:, bb * S:(bb + 1) * S],
                                        y_sb[:, b, :], ident[:S, :S])
                nc.vector.tensor_copy(yT[0:D, 384:768], ytp1[:, 0:384])

                st[j, "osb"] = osb_pool.tile([P, 6, C], BF16, tag="osb",
                                             name=f"osb{j}")
                st[j, "ops"] = [None] * 6
                st[j, "ocp"] = set()
                emit_outproj(j, (0, 1, 2, 3))

            # ---- stage A: max tree + eks + teq tail (slack: 1 full iter) ----
            if k < HL:
                eqk = st[k, "eqk"]
                ek_v = eqk[:, :, D:2 * D]
                mxB = small_pool.tile([S, 2, D], BF16, tag="mxB")
                mx2 = small_pool.tile([S, 2, D], BF16, tag="mx2")
                emx = small_pool.tile([S, D], F32, tag="emx")
                remx = small_pool.tile([S, D], F32, tag="remx")
                nc.vector.tensor_max(mxB[:], ek_v[:, 4:6, :], ek_v[:, 6:8, :])
                nc.vector.tensor_max(mx2[:], st[k, "mxA"][:], mxB[:])
                nc.vector.tensor_max(
                    emx[:], mx2[:, 0:1, :].rearrange("p o d -> p (o d)"),
                    mx2[:, 1:2, :].rearrange("p o d -> p (o d)"))
                nc.vector.reciprocal_approx_fast(remx[:], emx[:])

                eks = eks_pool.tile([S, 2, B, D], BF16, tag="eks", name=f"eks{k}")
                st[k, "eks"] = eks
                nc.vector.tensor_mul(
                    eks[:, 1, :, :], ek_v[:],
                    remx[:, :].rearrange("p (o d) -> p o d", o=1).broadcast_to([S, B, D]))
                nc.vector.tensor_mul(eks[:, 0, :, :], eks[:, 1, :, :],
                                     st[k, "v"][:])
                teq = teq_pool.tile([S, B, D], BF16, tag="teq", name=f"teq{k}")
                nc.vector.tensor_mul(teq[:], eqk[:, :, 0:D], eqb8[:])
                st[k, "teq"] = teq

            # ---- stage B: evacuate out-proj 0-3 ----
            if 0 <= j < HL:
                emit_ocopy(j, 0, "act")
                emit_ocopy(j, 1, "act")
                emit_ocopy(j, 2, "act")
                emit_ocopy(j, 3, "act")

            if 0 <= j3:
                for key in ("eqk", "v", "teq", "eks", "mxA", "osb", "ops",
                            "ocp"):
                    st.pop((j3, key), None)
                xT_tiles.pop(j3, None)

    if not nc.is_finalized():
        nc.finalize()
    return nc


def _make_blob(wq_w, wq_b, wk_w, wk_b, wv_w, wv_b, out_w, out_b, w_bias_table):
    blob = np.zeros((P, BLOB_F), dtype=np.float32)
    # wq negated so exp(-q) is a scale=1 activation
    for j, w in enumerate([-np.asarray(wq_w), wk_w, wv_w]):  # wqkvT [128, 4, 192]
        w = np.asarray(w)
        for ch in range(4):
            blob[:, ch * 192 + j * D:(ch * 192 + (j + 1) * D)] = \
                w[:, ch * P:(ch + 1) * P].T
    blob[0:D, 768:1280] = np.asarray(out_w).T        # owT
    blob[D, 768:1280] = out_b
    blob[0:S, 1280:1376] = np.exp(np.asarray(w_bias_table)).T
    # wk_b cancels exactly in exp(k - max_b k); wq_b folded via exp(-wq_b),
    # wv_b added to v after the projection.
    blob[0:S, 1376:1888] = np.tile(np.exp(-np.asarray(wq_b)), B)[None, :]
    blob[0:S, 1888:2400] = np.tile(np.asarray(wv_b), B)[None, :]
    return blob


def _build_in_maps(x, wq_w, wq_b, wk_w, wk_b, wv_w, wv_b, out_w, out_b,
                   w_bias_table):
    blob = _make_blob(wq_w, wq_b, wk_w, wk_b, wv_w, wv_b, out_w, out_b,
                      w_bias_table).astype(BF16NP)
    xbf = np.asarray(x).astype(BF16NP)               # [8, 96, 96, 512]
    in_maps = []
    for i in range(NCORES):
        # [B, HL, S, C] -> [HL, C, B, S] -> [HL, 4, 128, B*S]
        # -> [HL, 128, 4, B*S]  (c-within-chunk on partitions)
        xs = xbf[:, i * HL:(i + 1) * HL].transpose(1, 3, 0, 2)
        xs = xs.reshape(HL, 4, P, NPOS).transpose(0, 2, 1, 3)
        in_maps.append({
            "wblob": blob,
            "xt": np.ascontiguousarray(xs),
        })
    return in_maps


def kernel(x, wq_w, wq_b, wk_w, wk_b, wv_w, wv_b, out_w, out_b, w_bias_table):
    if "nc" not in _NC_CACHE:
        _NC_CACHE["nc"] = build_kernel()
    nc = _NC_CACHE["nc"]

    in_maps = _build_in_maps(x, wq_w, wq_b, wk_w, wk_b, wv_w, wv_b,
                             out_w, out_b, w_bias_table)
    res = run_bass_kernel_spmd(nc, in_maps, list(range(NCORES)))
    # per-core out: [HL, P, 6, C] bf16, pos = t*128 + p = b*96 + w
    outs = []
    for i in range(NCORES):
        a = np.asarray(res.results[i]["out"])        # [HL, 128, 6, 512]
        a = a.transpose(0, 2, 1, 3).reshape(HL, B, S, C).transpose(1, 0, 2, 3)
        outs.append(a)
    return np.concatenate(outs, axis=1).astype(np.float32)
